# revision 37
# baseline (speedup 1.0000x reference)
"""Trainium2 Bass kernel for SimCLR NT-Xent contrastive loss.

Math (reference): normalize rows of z_i, z_j -> z_ij = concat; sim = (z_ij @ z_ij.T)/t;
loss_m = -cos_m/t + log(sum_n exp(sim_mn) - exp(sim_mm)); return mean(loss).

Sharding: each of the 8 cores receives the full [8192,128] embedding matrix
*rotated* so that its own 1024-row block comes first (host-side np.roll = pure
data movement).  The per-core program is then position-independent: it
normalizes all rows, transposes to [D, rows] layout, computes its 8x16 block-row
of the similarity matrix via PE matmuls, exponentiates with the ACT engine
(accum_out gives row sums for free), and emits per-row losses.  The host
gathers the 8x[128,8] per-row losses and takes the mean.

Key numerics choices (all validated against the fp32 reference):
 - matmul operands in bf16 (PE full rate); accumulation in fp32 PSUM.
 - 1/||z|| computed as exp(-0.5*ln(sumsq)) so every ACT call (Ln/Exp) lives in
   one table set (natural_log_exp_and_others) -> one ACT_TABLE_LOAD.
 - the diagonal term exp(sim_mm) is the constant e^2 up to ~1e-3 relative;
   its contribution to the denominator (~8300) is ~1e-3*7.4/8300 ~ 1e-6.
"""

from contextlib import ExitStack

import numpy as np

import concourse.bass as bass
import concourse.mybir as mybir
import concourse.tile as tile
from concourse.bass_utils import run_bass_kernel_spmd


P = 128  # SBUF partitions
D = 128  # embedding dim
TEMP = 0.5
INV_TEMP = 1.0 / TEMP
E2 = float(np.exp(np.float32(2.0)))  # exp(sim_mm) = exp(||zn||^2 / t) = e^2

N_CORES = 8
FULL_R = 8192          # 2N rows
FULL_RC = FULL_R // N_CORES  # rows per core


def emit(tc, z, out, R, RC, CH):
    """Emit the per-core program.

    z:   DRAM [R, D] f32, rotated so this core's RC rows come first.
    out: DRAM [P, RC//P] f32 per-row losses (col m = m-th 128-row tile).
    CH:  ACT/PSUM chunk width (multiple of 512, CH*4B*P <= 8 PSUM banks).
    """
    nc = tc.nc
    f32 = mybir.dt.float32
    bf16 = mybir.dt.bfloat16
    AF = mybir.ActivationFunctionType
    ALU = mybir.AluOpType
    X = mybir.AxisListType.X

    T = R // P          # row tiles
    MT = RC // P        # row tiles owned by this core
    assert CH % 512 == 0 and R % 512 == 0 and T % 2 == 0

    from concourse.tile_rust import add_dep_helper, annotate_deps

    def dep_nop(eng, *aps):
        """Sequencer nop that 'reads' aps (dep-annotated like Tile's own
        critical-section helper).  Used to advance the SP sequencer's
        observed clock one semaphore at a time, so the end-of-program Drain
        needs no waits of its own (its CTRL struct has few sync-wait
        slots)."""
        n = eng.nop(hint="dep").ins
        n.ins = [eng.lower_ap(a) for a in aps]
        annotate_deps(tc.dep_state, n, tc.shadow_memory, tc._rust_ctx,
                      nc.inst_map)

    ctx = ExitStack()
    with ctx:
        consts = ctx.enter_context(tc.tile_pool(name="consts", bufs=1))
        big = ctx.enter_context(tc.tile_pool(name="big", bufs=1))
        work = ctx.enter_context(tc.tile_pool(name="work", bufs=3))

        # The transpose identity rides in as the last 128 rows of z (appended
        # by kernel()): no gpsimd-built identity -> Pool engine stays idle ->
        # one fewer semaphore in the end-of-program Drain (its CTRL struct
        # has few sync-wait slots).
        ident = consts.tile([P, P], bf16)
        zero_col = consts.tile([P, 1], f32)
        nc.vector.memset(zero_col, 0.0)
        neg_e2 = consts.tile([P, 1], f32)
        nc.vector.memset(neg_e2, -E2)

        zraw = big.tile([P, T + 1, D], f32)  # [p, t, d] = z[t*128+p, d]; tile T = identity
        zn = big.tile([P, T, D], bf16)     # normalized rows, bf16
        zT = big.tile([P, R], bf16)        # transposed: [d, r]
        ssum = big.tile([P, T], f32)       # per-row sum of squares
        inv = big.tile([P, T], f32)        # 1/sqrt(ssum)
        EX = big.tile([P, MT], f32)        # per-row exp-sums
        cosb = big.tile([P, MT], f32)      # positive-pair cosines

        zr = z.rearrange("(t p) d -> p t d", p=P)

        # --- Phase 1: load + normalize ---
        # At most 2 input DMAs: the final store then lands on a fresh DMAHW
        # lane (lane reuse would overflow the DMA struct's single sync-wait
        # slot), and the end-of-program Drain waits on few enough semaphores
        # to fit its CTRL struct.
        if T % 32 == 0 and T > 32:
            dma_bounds = [(0, 32), (32, T + 1)]
            GT = 32
        else:
            dma_bounds = [(0, T + 1)]
            GT = T
        for a, b in dma_bounds:
            nc.sync.dma_start(out=zraw[:, a:b, :], in_=zr[:, a:b, :])
        for g in range(T // GT):
            t0 = g * GT
            for t in range(t0, t0 + GT):
                sq = work.tile([P, D], f32, tag="sqdump")
                nc.vector.tensor_mul(sq, zraw[:, t, :], zraw[:, t, :])
                nc.vector.tensor_reduce(
                    out=ssum[:, t:t + 1], in_=sq, axis=X, op=ALU.add)
            # inv = exp(-0.5 * ln(ssum)) -- stays inside the ln/exp table set
            nc.scalar.activation(out=inv[:, t0:t0 + GT], in_=ssum[:, t0:t0 + GT],
                                 func=AF.Ln, bias=zero_col, scale=1.0)
            nc.scalar.activation(out=inv[:, t0:t0 + GT], in_=inv[:, t0:t0 + GT],
                                 func=AF.Exp, bias=zero_col, scale=-0.5)
            for t in range(t0, t0 + GT):
                nc.vector.tensor_scalar_mul(
                    out=zn[:, t, :], in0=zraw[:, t, :], scalar1=inv[:, t:t + 1])

        # --- positive-pair cosines: rows m*128+p pair with rows R/2 + m*128+p ---
        for m in range(MT):
            dump = work.tile([P, D], f32, tag="cosdump")
            nc.vector.tensor_mul(dump, zn[:, m, :], zn[:, T // 2 + m, :])
            nc.vector.tensor_reduce(
                out=cosb[:, m:m + 1], in_=dump, axis=X, op=ALU.add)

        # --- Phase 2 + 3: transposes, then block-row of exp(sim) ---
        # PSUM budget: ptr 2x[P,P] = 2 banks, pmm 2x[P,1536] = 6 banks.
        # Pools coexist (no released-zone overlap deps, which would add
        # same-engine PE waits that overflow the MM struct's 1 wait slot).
        ptr = ctx.enter_context(tc.tile_pool(name="ptr", bufs=2, space="PSUM"))
        pmm = ctx.enter_context(tc.tile_pool(name="pmm", bufs=2, space="PSUM"))
        nc.vector.tensor_copy(out=ident, in_=zraw[:, T, :])  # f32 -> bf16
        for t in range(T):
            pt = ptr.tile([P, P], bf16, name="pt")
            nc.tensor.transpose(pt, zn[:, t, :], ident)
            nc.vector.tensor_copy(out=zT[:, t * P:(t + 1) * P], in_=pt)

        # Dummy PE op whose single DVE wait covers ALL zT copies (DVE sem is
        # monotone), so every subsequent matmul carries at most the ACT wait.
        pt_d = ptr.tile([P, P], bf16, name="pt_d", tag="pt")
        nc.tensor.transpose(pt_d, zT[:, R - P:R], ident)

        # Chunk schedule: ragged [1536 x 5, 512] per block-row (R = 8192).
        chunks = []
        off = 0
        while off < R:
            w = min(CH, R - off)
            chunks.append((off, w))
            off += w
        NCHR = len(chunks)

        # Scratch sink for the tiny ACT absorber ops (disjoint columns -> no
        # WAW deps between them).
        tinyt = big.tile([P, MT * NCHR * 4], f32)

        esums_list = []
        for m in range(MT):
            esums = work.tile([P, NCHR], f32, tag="esums", bufs=MT)
            esums_list.append(esums)
            lhsT = zT[:, m * P:(m + 1) * P]
            for ci, (off, w) in enumerate(chunks):
                gc = m * NCHR + ci
                ps = pmm.tile([P, CH], f32, name="ps")
                # PE-side absorber: a bare LDWEIGHTS (no memory output, so no
                # WAW self-wait) reading the esums column written by the exp
                # that freed this PSUM slot two chunks ago.  It soaks up the
                # ACT wait so every real matmul below carries only its PE
                # self-wait — the MM ISA struct has a single sync-wait slot.
                # (bitcast to bf16: standalone f32 LDW fails walrus codegen;
                # the garbage weights are overwritten by the next matmul's
                # self-loading LDW.)
                if gc >= 2:
                    m2, c2 = divmod(gc - 2, NCHR)
                    ecol = esums_list[m2][:, c2:c2 + 1]
                    nc.tensor.ldweights(ecol.bitcast(bf16))
                for s in range(w // 512):
                    c0 = off + s * 512
                    last_mm = nc.tensor.matmul(
                        ps[:, s * 512:(s + 1) * 512],
                        lhsT, zT[:, c0:c0 + 512],
                        start=True, stop=True,
                    )
                # ACT-side absorber: discarded exp reading one column per
                # 512-segment soaks up the PE waits, so the real exp carries
                # only its ACT self-wait (ACTIVATION struct: 1 wait slot).
                nseg = w // 512
                nc.scalar.activation(
                    out=tinyt[:, gc * 4:gc * 4 + nseg],
                    in_=ps[:, 0:w:512], func=AF.Exp,
                    bias=zero_col, scale=1.0,
                )
                nc.scalar.activation(
                    out=ps[:, 0:w], in_=ps[:, 0:w], func=AF.Exp,
                    bias=zero_col, scale=INV_TEMP,
                    accum_out=esums[:, ci:ci + 1],
                )
            nc.vector.tensor_reduce(
                out=EX[:, m:m + 1], in_=esums, axis=X, op=ALU.add)

        # --- Phase 4: loss = ln(EX - e^2) - 2*cos ---
        lnden = work.tile([P, MT], f32, tag="lnden")
        nc.scalar.activation(out=lnden, in_=EX, func=AF.Ln,
                             bias=neg_e2, scale=1.0)
        lossv = work.tile([P, MT], f32, tag="lossv")
        # DVE-side absorber for the ACT->DVE handoff (STT struct: 1 slot).
        tiny2 = work.tile([P, 1], f32, tag="tiny2")
        nc.vector.tensor_copy(out=tiny2, in_=lnden[:, 0:1])
        nc.vector.scalar_tensor_tensor(
            out=lossv, in0=cosb, scalar=-INV_TEMP, in1=lnden,
            op0=ALU.mult, op1=ALU.add,
        )
        nc.sync.dma_start(out=out, in_=lossv)

        # Pre-absorb the final Drain's waits one semaphore at a time: each
        # nop carries a single wait, advancing SP's observed clock so the
        # end-of-program Drain (CTRL struct, few sync-wait slots) needs none.
        for a, b in dma_bounds:
            dep_nop(nc.sync, zraw[:, a:b, :])     # DMAHW lanes (inputs)
        dep_nop(nc.sync, lnden[:, :])             # ACT final tick
        dep_nop(nc.sync, lossv[:, :])             # DVE final tick
        dep_nop(nc.sync, out)                     # out-DMA completion
        # PE final tick: the last matmul's psum write is overwritten by the
        # exp, so no AP read can reach it -- add a direct dep edge instead.
        pe_nop = nc.sync.nop(hint="dep").ins
        add_dep_helper(pe_nop, last_mm.ins, True, "drain pre-absorb: PE")


def build(R=FULL_R, RC=FULL_RC, CH=1536):
    nc = bass.Bass("TRN2", target_bir_lowering=False, debug=False,
                   num_devices=R // RC)
    # Last 128 rows of z carry the transpose identity matrix.
    z = nc.dram_tensor("z", [R + P, D], mybir.dt.float32, kind="ExternalInput")
    out = nc.dram_tensor("out", [P, RC // P], mybir.dt.float32,
                         kind="ExternalOutput")
    with tile.TileContext(nc) as tc:
        emit(tc, z.ap(), out.ap(), R, RC, CH)
    return nc


_CACHE = {}


def kernel(z_i, z_j):
    z_i = np.ascontiguousarray(np.asarray(z_i, dtype=np.float32))
    z_j = np.ascontiguousarray(np.asarray(z_j, dtype=np.float32))
    assert z_i.shape == (FULL_R // 2, D) and z_j.shape == (FULL_R // 2, D)

    if "nc" not in _CACHE:
        _CACHE["nc"] = build()
    nc = _CACHE["nc"]

    z_all = np.concatenate([z_i, z_j], axis=0)  # [8192, 128]
    eye = np.eye(P, dtype=np.float32)
    in_maps = [
        {"z": np.ascontiguousarray(np.concatenate(
            [np.roll(z_all, -c * FULL_RC, axis=0), eye], axis=0))}
        for c in range(N_CORES)
    ]
    res = run_bass_kernel_spmd(nc, in_maps, core_ids=list(range(N_CORES)))
    total = 0.0
    for r in res.results:
        total += float(np.asarray(r["out"], dtype=np.float64).sum())
    return np.float32(total / FULL_R)
